# revision 12
# baseline (speedup 1.0000x reference)
"""Dense dot-product attention on 8 Trainium2 NeuronCores.

Problem: query/key/value [32, 2048, 64] fp32 -> softmax(Q K^T / 8) V.
Sharding: batch dim split 4-per-core across 8 cores (data parallel, no
collectives). Each core computes full attention for its 4 batches.

Per-batch dataflow (matmuls in f32r = full-rate fp32 PE mode, ~1.6e-4):
  1. DMA Q,K natural [2048,64]; PE-transpose (col-packed pairs via
     tile_position) -> Q^T,K^T duplicated into both partition halves
     [128, 2048] f32r in SBUF.
  2. S^T[k,q] = K^T.T @ Q^T, two k-tiles concurrently via row-packed
     matmul pairs (strip rows 0-63 / 64-127), PSUM [128k, 1024q] blocks.
  3. exp on ScalarE straight out of PSUM (scale=1/8 folded in), f32r out.
     No max-subtraction: scores ~ N(0,1), exp cannot overflow.
  4. P@V via PE with lhsT = [V | ones] [128k, 65]: accumulates
     out^T [65, q] over the 16 k-tiles; row 64 = softmax denominator.
  5. PE-transpose out^T chunks -> [128q, 65], DVE reciprocal of col 64,
     row-scale cols 0..63, DMA out.

The PE HAM clock-gate only counts bf16-class matmul activity: an
all-f32r kernel runs at 1.2 GHz forever. A sparse carpet of bf16
"warmer" matmuls (~1 per 3 us of PE work) keeps the clock at 2.4 GHz.
"""

import numpy as np

B, L, D = 32, 2048, 64
NCORES = 8
B_SH = B // NCORES          # 4 batches per core
LT = L // 128               # 16 k/l tiles of 128
NQH = 2                     # q processed in halves of 1024
QHW = L // NQH              # 1024
SCALE = 1.0 / np.sqrt(np.float32(D))  # 0.125

_cached = {}


def _build():
    import concourse.bacc as bacc
    import concourse.tile as tile
    from concourse import mybir
    from concourse.masks import make_identity

    f32 = mybir.dt.float32
    f32r = mybir.dt.float32r
    bf16 = mybir.dt.bfloat16
    Exp = mybir.ActivationFunctionType.Exp

    nc = bacc.Bacc("TRN2", target_bir_lowering=False, debug=False)

    q_d = nc.dram_tensor("query", [B_SH, L, D], f32, kind="ExternalInput")
    k_d = nc.dram_tensor("key", [B_SH, L, D], f32, kind="ExternalInput")
    v_d = nc.dram_tensor("value", [B_SH, L, D], f32, kind="ExternalInput")
    o_d = nc.dram_tensor("out", [B_SH, L, D], f32, kind="ExternalOutput")

    with tile.TileContext(nc) as tc:
        with (
            tc.tile_pool(name="consts", bufs=1) as consts,
            tc.tile_pool(name="nat", bufs=2) as nat,
            tc.tile_pool(name="vst", bufs=2) as vst,
            tc.tile_pool(name="qkt", bufs=2) as qkt,
            tc.tile_pool(name="vr", bufs=2) as vrp,
            tc.tile_pool(name="er", bufs=3) as erp,
            tc.tile_pool(name="pvsb", bufs=2) as pvsb,
            tc.tile_pool(name="oall", bufs=2) as oallp,
            tc.tile_pool(name="rz", bufs=4) as rzp,
            tc.tile_pool(name="sps", bufs=2, space="PSUM") as sps,
            tc.tile_pool(name="pvps", bufs=1, space="PSUM") as pvps,
            tc.tile_pool(name="trps", bufs=1, space="PSUM") as trps,
            tc.tile_pool(name="wps", bufs=1, space="PSUM") as wps,
        ):
            ident = consts.tile([128, 128], f32)
            make_identity(nc, ident)
            ones_col = consts.tile([128, LT, 1], f32)
            nc.vector.memset(ones_col, 1.0)

            wsrc = consts.tile([128, 512], bf16)
            nc.vector.memset(wsrc, 1.0)
            wp = wps.tile([128, 512], f32)

            def warmer(n=512):
                nc.tensor.matmul(wp[:, 0:n], wsrc[:, 0:128], wsrc[:, 0:n],
                                 start=True, stop=True, skip_group_check=True)

            # per-batch persistent tiles
            qkT = {}   # b -> (qT, kT) [128, 2048] f32r, both halves identical
            v_r = {}   # b -> [128, 16, 65] f32r  (col 64 = 1.0)

            def prep_load(b):
                """DMA loads + V staging + tile allocation for batch b."""
                q_nat = nat.tile([128, LT, D], f32, tag="qnat")
                k_nat = nat.tile([128, LT, D], f32, tag="knat")
                nc.sync.dma_start(
                    out=q_nat, in_=q_d.ap()[b].rearrange("(t p) d -> p t d", p=128))
                nc.sync.dma_start(
                    out=k_nat, in_=k_d.ap()[b].rearrange("(t p) d -> p t d", p=128))

                qT = qkt.tile([128, L], f32r, tag="qT")
                kT = qkt.tile([128, L], f32r, tag="kT")

                v_stage = vst.tile([128, LT, D], f32, tag="vstage")
                nc.sync.dma_start(
                    out=v_stage, in_=v_d.ap()[b].rearrange("(t p) d -> p t d", p=128))
                vr = vrp.tile([128, LT, D + 1], f32r, tag="vr")
                nc.vector.tensor_copy(out=vr[:, :, 0:D], in_=v_stage)
                nc.vector.tensor_copy(out=vr[:, :, D:D + 1], in_=ones_col)

                qkT[b] = (qT, kT)
                v_r[b] = vr

                jobs = []
                for lt in range(LT):
                    for src, dst in ((q_nat, qT), (k_nat, kT)):
                        def tr_job(src=src, dst=dst, lt=lt):
                            tp = trps.tile([64, 128], f32, tag="tr")
                            nc.tensor.transpose(tp, src[:, lt, :], ident)
                            nc.vector.tensor_copy(
                                out=dst[0:64, lt * 128:(lt + 1) * 128], in_=tp)
                        jobs.append(tr_job)

                def dup_job():
                    # upper partition half copies for row-packed MMs
                    nc.sync.dma_start(out=qT[64:128, :], in_=qT[0:64, :])
                    nc.sync.dma_start(out=kT[64:128, :], in_=kT[0:64, :])
                jobs.append(dup_job)
                return jobs

            def main(b, next_jobs):
                qT, kT = qkT.pop(b)
                vr = v_r.pop(b)
                slot = 0
                for qh in range(NQH):
                    q0 = qh * QHW
                    pv = pvps.tile([D + 1, QHW], f32, tag="pv")
                    for kp in range(LT // 2):      # pairs of k-tiles
                        ka, kb = 2 * kp, 2 * kp + 1
                        warmer(256)
                        # interleave next batch's transposes into this
                        # batch's PE stream (2 per pair-slot in qh 0)
                        for _ in range(2):
                            if slot < len(next_jobs):
                                next_jobs[slot]()
                                slot += 1
                        s_a = sps.tile([128, QHW], f32, tag="s")
                        s_b = sps.tile([128, QHW], f32, tag="s")
                        for j in range(QHW // 512):
                            js = slice(j * 512, (j + 1) * 512)
                            qs = slice(q0 + j * 512, q0 + (j + 1) * 512)
                            nc.tensor.matmul(
                                s_a[:, js], kT[0:64, ka * 128:(ka + 1) * 128],
                                qT[0:64, qs], start=True, stop=True)
                            nc.tensor.matmul(
                                s_b[:, js], kT[64:128, kb * 128:(kb + 1) * 128],
                                qT[64:128, qs], start=True, stop=True)
                        for kt, s_ps in ((ka, s_a), (kb, s_b)):
                            e_r = erp.tile([128, QHW], f32r, tag="e")
                            nc.scalar.activation(out=e_r, in_=s_ps, func=Exp,
                                                 scale=float(SCALE))
                            warmer(256)
                            for j in range(QHW // 512):
                                js = slice(j * 512, (j + 1) * 512)
                                nc.tensor.matmul(
                                    pv[:, js], vr[:, kt, :], e_r[:, js],
                                    start=(kt == 0), stop=(kt == LT - 1))

                    pv_sb = pvsb.tile([D + 1, QHW], f32, tag="pvsb")
                    nc.vector.tensor_copy(out=pv_sb, in_=pv)

                    o_all = oallp.tile([128, QHW // 128, D], f32, tag="oall")
                    for qt in range(QHW // 128):
                        ot = trps.tile([128, D + 1], f32, tag="tr")
                        nc.tensor.transpose(
                            ot, pv_sb[:, qt * 128:(qt + 1) * 128],
                            ident[0:D + 1, 0:D + 1])
                        rz = rzp.tile([128, 1], f32, tag="rz")
                        nc.vector.reciprocal(out=rz, in_=ot[:, D:D + 1])
                        nc.vector.tensor_scalar_mul(
                            out=o_all[:, qt, :], in0=ot[:, 0:D], scalar1=rz)
                    nc.sync.dma_start(
                        out=o_d.ap()[b, q0:q0 + QHW, :].rearrange(
                            "(t p) d -> p t d", p=128),
                        in_=o_all)
                while slot < len(next_jobs):
                    next_jobs[slot]()
                    slot += 1

            # batch 0 prologue: loads + transposes, with warmers woven in
            # (also serves as the initial clock warm-up burst)
            jobs0 = prep_load(0)
            for i, job in enumerate(jobs0):
                job()
                if i % 2 == 0:
                    warmer()
            for _ in range(8):
                warmer()
            for b in range(B_SH):
                nxt = prep_load(b + 1) if b + 1 < B_SH else []
                main(b, nxt)

    nc.finalize()
    return nc


def _get_nc():
    if "nc" not in _cached:
        _cached["nc"] = _build()
    return _cached["nc"]


def kernel(query, key, value):
    from concourse.bass_utils import run_bass_kernel_spmd

    nc = _get_nc()
    query = np.ascontiguousarray(query, dtype=np.float32)
    key = np.ascontiguousarray(key, dtype=np.float32)
    value = np.ascontiguousarray(value, dtype=np.float32)

    in_maps = []
    for c in range(NCORES):
        sl = slice(c * B_SH, (c + 1) * B_SH)
        in_maps.append({
            "query": query[sl], "key": key[sl], "value": value[sl]})

    res = run_bass_kernel_spmd(nc, in_maps, core_ids=list(range(NCORES)))
    out = np.concatenate([r["out"] for r in res.results], axis=0)
    return out


# revision 13
# speedup vs baseline: 1.0423x; 1.0423x over previous
"""Dense dot-product attention on 8 Trainium2 NeuronCores.

Problem: query/key/value [32, 2048, 64] fp32 -> softmax(Q K^T / 8) V.
Sharding: batch dim split 4-per-core across 8 cores (data parallel, no
collectives). Each core computes full attention for its 4 batches.

Per-batch dataflow (matmuls in f32r = full-rate fp32 PE mode, ~1.6e-4):
  1. DMA Q,K natural [2048,64]; PE-transpose (col-packed pairs via
     tile_position) -> Q^T,K^T duplicated into both partition halves
     [128, 2048] f32r in SBUF.
  2. S^T[k,q] = K^T.T @ Q^T, two k-tiles concurrently via row-packed
     matmul pairs (strip rows 0-63 / 64-127), PSUM [128k, 1024q] blocks.
  3. exp on ScalarE straight out of PSUM (scale=1/8 folded in), f32r out.
     No max-subtraction: scores ~ N(0,1), exp cannot overflow.
  4. P@V via PE with lhsT = [V | ones] [128k, 65]: accumulates
     out^T [65, q] over the 16 k-tiles; row 64 = softmax denominator.
  5. PE-transpose out^T chunks -> [128q, 65], DVE reciprocal of col 64,
     row-scale cols 0..63, DMA out.

The PE HAM clock-gate only counts bf16-class matmul activity: an
all-f32r kernel runs at 1.2 GHz forever. A sparse carpet of bf16
"warmer" matmuls (~1 per 3 us of PE work) keeps the clock at 2.4 GHz.
"""

import numpy as np

B, L, D = 32, 2048, 64
NCORES = 8
B_SH = B // NCORES          # 4 batches per core
LT = L // 128               # 16 k/l tiles of 128
NQH = 2                     # q processed in halves of 1024
QHW = L // NQH              # 1024
SCALE = 1.0 / np.sqrt(np.float32(D))  # 0.125

_cached = {}


def _build():
    import concourse.bacc as bacc
    import concourse.tile as tile
    from concourse import mybir
    from concourse.masks import make_identity
    from concourse.tile import add_dep_helper

    f32 = mybir.dt.float32
    f32r = mybir.dt.float32r
    bf16 = mybir.dt.bfloat16
    Exp = mybir.ActivationFunctionType.Exp

    nc = bacc.Bacc("TRN2", target_bir_lowering=False, debug=False)

    q_d = nc.dram_tensor("query", [B_SH, L, D], f32, kind="ExternalInput")
    k_d = nc.dram_tensor("key", [B_SH, L, D], f32, kind="ExternalInput")
    v_d = nc.dram_tensor("value", [B_SH, L, D], f32, kind="ExternalInput")
    o_d = nc.dram_tensor("out", [B_SH, L, D], f32, kind="ExternalOutput")

    with tile.TileContext(nc) as tc:
        with (
            tc.tile_pool(name="consts", bufs=1) as consts,
            tc.tile_pool(name="nat", bufs=2) as nat,
            tc.tile_pool(name="vst", bufs=2) as vst,
            tc.tile_pool(name="qkt", bufs=2) as qkt,
            tc.tile_pool(name="vr", bufs=2) as vrp,
            tc.tile_pool(name="er", bufs=3) as erp,
            tc.tile_pool(name="pvsb", bufs=2) as pvsb,
            tc.tile_pool(name="oall", bufs=2) as oallp,
            tc.tile_pool(name="rz", bufs=4) as rzp,
            tc.tile_pool(name="sps", bufs=2, space="PSUM") as sps,
            tc.tile_pool(name="pvps", bufs=1, space="PSUM") as pvps,
            tc.tile_pool(name="trps", bufs=1, space="PSUM") as trps,
            tc.tile_pool(name="wps", bufs=1, space="PSUM") as wps,
        ):
            ident = consts.tile([128, 128], f32)
            make_identity(nc, ident)
            ones_col = consts.tile([128, LT, 1], f32)
            nc.vector.memset(ones_col, 1.0)

            wsrc = consts.tile([128, 512], bf16)
            nc.vector.memset(wsrc, 1.0)
            wp = wps.tile([128, 512], f32)

            def warmer(n=512, anchor=None):
                w = nc.tensor.matmul(wp[:, 0:n], wsrc[:, 0:128], wsrc[:, 0:n],
                                     start=True, stop=True,
                                     skip_group_check=True)
                if anchor is not None:
                    add_dep_helper(w.ins, anchor.ins, sync=False,
                                   reason="pin clock warmer in PE stream")
                return w

            # per-batch persistent tiles
            qkT = {}   # b -> (qT, kT) [128, 2048] f32r, both halves identical
            v_r = {}   # b -> [128, 16, 65] f32r  (col 64 = 1.0)

            def prep_load(b):
                """DMA loads + V staging + tile allocation for batch b."""
                q_nat = nat.tile([128, LT, D], f32, tag="qnat")
                k_nat = nat.tile([128, LT, D], f32, tag="knat")
                nc.sync.dma_start(
                    out=q_nat, in_=q_d.ap()[b].rearrange("(t p) d -> p t d", p=128))
                nc.sync.dma_start(
                    out=k_nat, in_=k_d.ap()[b].rearrange("(t p) d -> p t d", p=128))

                qT = qkt.tile([128, L], f32r, tag="qT")
                kT = qkt.tile([128, L], f32r, tag="kT")

                v_stage = vst.tile([128, LT, D], f32, tag="vstage")
                nc.sync.dma_start(
                    out=v_stage, in_=v_d.ap()[b].rearrange("(t p) d -> p t d", p=128))
                vr = vrp.tile([128, LT, D + 1], f32r, tag="vr")
                nc.vector.tensor_copy(out=vr[:, :, 0:D], in_=v_stage)
                nc.vector.tensor_copy(out=vr[:, :, D:D + 1], in_=ones_col)

                qkT[b] = (qT, kT)
                v_r[b] = vr

                jobs = []
                for lt in range(LT):
                    for src, dst in ((q_nat, qT), (k_nat, kT)):
                        def tr_job(src=src, dst=dst, lt=lt):
                            tp = trps.tile([64, 128], f32, tag="tr")
                            tr = nc.tensor.transpose(tp, src[:, lt, :], ident)
                            nc.vector.tensor_copy(
                                out=dst[0:64, lt * 128:(lt + 1) * 128], in_=tp)
                            return tr
                        jobs.append(tr_job)

                def dup_job():
                    # upper partition half copies for row-packed MMs
                    nc.sync.dma_start(out=qT[64:128, :], in_=qT[0:64, :])
                    nc.sync.dma_start(out=kT[64:128, :], in_=kT[0:64, :])
                    return None
                jobs.append(dup_job)
                return jobs

            def main(b, next_jobs):
                qT, kT = qkT.pop(b)
                vr = v_r.pop(b)
                slot = 0
                for qh in range(NQH):
                    q0 = qh * QHW
                    pv = pvps.tile([D + 1, QHW], f32, tag="pv")
                    for kp in range(LT // 2):      # pairs of k-tiles
                        ka, kb = 2 * kp, 2 * kp + 1
                        # interleave next batch's transposes into this
                        # batch's PE stream (2 per pair-slot)
                        for _ in range(2):
                            if slot < len(next_jobs):
                                tr = next_jobs[slot]()
                                if tr is not None:
                                    warmer(256, anchor=tr)
                                slot += 1
                        s_a = sps.tile([128, QHW], f32, tag="s")
                        s_b = sps.tile([128, QHW], f32, tag="s")
                        last_s = None
                        for j in range(QHW // 512):
                            js = slice(j * 512, (j + 1) * 512)
                            qs = slice(q0 + j * 512, q0 + (j + 1) * 512)
                            nc.tensor.matmul(
                                s_a[:, js], kT[0:64, ka * 128:(ka + 1) * 128],
                                qT[0:64, qs], start=True, stop=True)
                            last_s = nc.tensor.matmul(
                                s_b[:, js], kT[64:128, kb * 128:(kb + 1) * 128],
                                qT[64:128, qs], start=True, stop=True)
                        warmer(256, anchor=last_s)
                        for kt, s_ps in ((ka, s_a), (kb, s_b)):
                            e_r = erp.tile([128, QHW], f32r, tag="e")
                            nc.scalar.activation(out=e_r, in_=s_ps, func=Exp,
                                                 scale=float(SCALE))
                            last_pv = None
                            for j in range(QHW // 512):
                                js = slice(j * 512, (j + 1) * 512)
                                last_pv = nc.tensor.matmul(
                                    pv[:, js], vr[:, kt, :], e_r[:, js],
                                    start=(kt == 0), stop=(kt == LT - 1))
                            warmer(256, anchor=last_pv)

                    pv_sb = pvsb.tile([D + 1, QHW], f32, tag="pvsb")
                    nc.vector.tensor_copy(out=pv_sb, in_=pv)

                    o_all = oallp.tile([128, QHW // 128, D], f32, tag="oall")
                    for qt in range(QHW // 128):
                        ot = trps.tile([128, D + 1], f32, tag="tr")
                        nc.tensor.transpose(
                            ot, pv_sb[:, qt * 128:(qt + 1) * 128],
                            ident[0:D + 1, 0:D + 1])
                        rz = rzp.tile([128, 1], f32, tag="rz")
                        nc.vector.reciprocal(out=rz, in_=ot[:, D:D + 1])
                        nc.vector.tensor_scalar_mul(
                            out=o_all[:, qt, :], in0=ot[:, 0:D], scalar1=rz)
                    nc.sync.dma_start(
                        out=o_d.ap()[b, q0:q0 + QHW, :].rearrange(
                            "(t p) d -> p t d", p=128),
                        in_=o_all)
                while slot < len(next_jobs):
                    next_jobs[slot]()
                    slot += 1

            # batch 0 prologue: loads + transposes, with warmers woven in
            # (also serves as the initial clock warm-up burst)
            jobs0 = prep_load(0)
            for i, job in enumerate(jobs0):
                tr = job()
                if i % 2 == 0 and tr is not None:
                    warmer(256, anchor=tr)
            for _ in range(8):
                warmer()
            for b in range(B_SH):
                nxt = prep_load(b + 1) if b + 1 < B_SH else []
                main(b, nxt)

    nc.finalize()
    return nc


def _get_nc():
    if "nc" not in _cached:
        _cached["nc"] = _build()
    return _cached["nc"]


def kernel(query, key, value):
    from concourse.bass_utils import run_bass_kernel_spmd

    nc = _get_nc()
    query = np.ascontiguousarray(query, dtype=np.float32)
    key = np.ascontiguousarray(key, dtype=np.float32)
    value = np.ascontiguousarray(value, dtype=np.float32)

    in_maps = []
    for c in range(NCORES):
        sl = slice(c * B_SH, (c + 1) * B_SH)
        in_maps.append({
            "query": query[sl], "key": key[sl], "value": value[sl]})

    res = run_bass_kernel_spmd(nc, in_maps, core_ids=list(range(NCORES)))
    out = np.concatenate([r["out"] for r in res.results], axis=0)
    return out


# revision 14
# speedup vs baseline: 1.0930x; 1.0486x over previous
"""Dense dot-product attention on 8 Trainium2 NeuronCores.

Problem: query/key/value [32, 2048, 64] fp32 -> softmax(Q K^T / 8) V.
Sharding: batch dim split 4-per-core across 8 cores (data parallel, no
collectives). Each core computes full attention for its 4 batches.

Per-batch dataflow (matmuls in f32r = full-rate fp32 PE mode, ~1.6e-4):
  1. DMA Q,K natural [2048,64]; PE-transpose (col-packed pairs via
     tile_position) -> Q^T,K^T duplicated into both partition halves
     [128, 2048] f32r in SBUF.
  2. S^T[k,q] = K^T.T @ Q^T, two k-tiles concurrently via row-packed
     matmul pairs (strip rows 0-63 / 64-127), PSUM [128k, 1024q] blocks.
  3. exp on ScalarE straight out of PSUM (scale=1/8 folded in), f32r out.
     No max-subtraction: scores ~ N(0,1), exp cannot overflow.
  4. P@V via PE with lhsT = [V | ones] [128k, 65]: accumulates
     out^T [65, q] over the 16 k-tiles; row 64 = softmax denominator.
  5. PE-transpose out^T chunks -> [128q, 65], DVE reciprocal of col 64,
     row-scale cols 0..63, DMA out.

The PE HAM clock-gate only counts bf16-class matmul activity: an
all-f32r kernel runs at 1.2 GHz forever. A sparse carpet of bf16
"warmer" matmuls (~1 per 3 us of PE work) keeps the clock at 2.4 GHz.
"""

import numpy as np

B, L, D = 32, 2048, 64
NCORES = 8
B_SH = B // NCORES          # 4 batches per core
LT = L // 128               # 16 k/l tiles of 128
NQH = 2                     # q processed in halves of 1024
QHW = L // NQH              # 1024
SCALE = 1.0 / np.sqrt(np.float32(D))  # 0.125

_cached = {}


def _build():
    import concourse.bacc as bacc
    import concourse.tile as tile
    from concourse import mybir
    from concourse.masks import make_identity
    from concourse.tile import add_dep_helper

    f32 = mybir.dt.float32
    f32r = mybir.dt.float32r
    bf16 = mybir.dt.bfloat16
    Exp = mybir.ActivationFunctionType.Exp

    nc = bacc.Bacc("TRN2", target_bir_lowering=False, debug=False)

    q_d = nc.dram_tensor("query", [B_SH, L, D], f32, kind="ExternalInput")
    k_d = nc.dram_tensor("key", [B_SH, L, D], f32, kind="ExternalInput")
    v_d = nc.dram_tensor("value", [B_SH, L, D], f32, kind="ExternalInput")
    o_d = nc.dram_tensor("out", [B_SH, L, D], f32, kind="ExternalOutput")

    with tile.TileContext(nc) as tc:
        with (
            tc.tile_pool(name="consts", bufs=1) as consts,
            tc.tile_pool(name="nat", bufs=2) as nat,
            tc.tile_pool(name="vst", bufs=2) as vst,
            tc.tile_pool(name="qkt", bufs=2) as qkt,
            tc.tile_pool(name="vr", bufs=2) as vrp,
            tc.tile_pool(name="er", bufs=3) as erp,
            tc.tile_pool(name="pvsb", bufs=2) as pvsb,
            tc.tile_pool(name="oall", bufs=2) as oallp,
            tc.tile_pool(name="rz", bufs=4) as rzp,
            tc.tile_pool(name="sps", bufs=2, space="PSUM") as sps,
            tc.tile_pool(name="pvps", bufs=1, space="PSUM") as pvps,
            tc.tile_pool(name="trps", bufs=1, space="PSUM") as trps,
            tc.tile_pool(name="wps", bufs=1, space="PSUM") as wps,
        ):
            ident = consts.tile([128, 128], f32)
            make_identity(nc, ident)
            wsrc = consts.tile([128, 512], bf16)
            nc.vector.memset(wsrc, 1.0)
            wp = wps.tile([128, 512], f32)

            def warmer(n=512, anchor=None):
                w = nc.tensor.matmul(wp[:, 0:n], wsrc[:, 0:128], wsrc[:, 0:n],
                                     start=True, stop=True,
                                     skip_group_check=True)
                if anchor is not None:
                    add_dep_helper(w.ins, anchor.ins, sync=False,
                                   reason="pin clock warmer in PE stream")
                return w

            # per-batch persistent tiles
            qkT = {}   # b -> (qT, kT) [128, 2048] f32r, both halves identical
            v_r = {}   # b -> [128, 16, 65] f32r  (col 64 = 1.0)

            def prep_load(b):
                """DMA loads + V staging + tile allocation for batch b."""
                q_nat = nat.tile([128, LT, D], f32, tag="qnat")
                k_nat = nat.tile([128, LT, D], f32, tag="knat")
                nc.sync.dma_start(
                    out=q_nat, in_=q_d.ap()[b].rearrange("(t p) d -> p t d", p=128))
                nc.sync.dma_start(
                    out=k_nat, in_=k_d.ap()[b].rearrange("(t p) d -> p t d", p=128))

                qT = qkt.tile([128, L], f32r, tag="qT")
                kT = qkt.tile([128, L], f32r, tag="kT")

                v_stage = vst.tile([128, LT, D], f32, tag="vstage")
                nc.sync.dma_start(
                    out=v_stage, in_=v_d.ap()[b].rearrange("(t p) d -> p t d", p=128))
                vr = vrp.tile([128, LT, D + 1], bf16, tag="vr")
                nc.vector.tensor_copy(out=vr[:, :, 0:D], in_=v_stage)
                nc.vector.memset(vr[:, :, D:D + 1], 1.0)

                qkT[b] = (qT, kT)
                v_r[b] = vr

                jobs = []
                for lt in range(LT):
                    for src, dst in ((q_nat, qT), (k_nat, kT)):
                        def tr_job(src=src, dst=dst, lt=lt):
                            tp = trps.tile([64, 128], f32, tag="tr")
                            tr = nc.tensor.transpose(tp, src[:, lt, :], ident)
                            nc.vector.tensor_copy(
                                out=dst[0:64, lt * 128:(lt + 1) * 128], in_=tp)
                            return tr
                        jobs.append(tr_job)

                def dup_job():
                    # upper partition half copies for row-packed MMs
                    nc.sync.dma_start(out=qT[64:128, :], in_=qT[0:64, :])
                    nc.sync.dma_start(out=kT[64:128, :], in_=kT[0:64, :])
                    return None
                jobs.append(dup_job)
                return jobs

            def main(b, next_jobs):
                qT, kT = qkT.pop(b)
                vr = v_r.pop(b)
                slot = 0
                for qh in range(NQH):
                    q0 = qh * QHW
                    pv = pvps.tile([D + 1, QHW], f32, tag="pv")
                    for kp in range(LT // 2):      # pairs of k-tiles
                        ka, kb = 2 * kp, 2 * kp + 1
                        # interleave next batch's transposes into this
                        # batch's PE stream (2 per pair-slot)
                        for _ in range(2):
                            if slot < len(next_jobs):
                                tr = next_jobs[slot]()
                                if tr is not None:
                                    warmer(256, anchor=tr)
                                slot += 1
                        s_a = sps.tile([128, QHW], f32, tag="s")
                        s_b = sps.tile([128, QHW], f32, tag="s")
                        last_s = None
                        for j in range(QHW // 512):
                            js = slice(j * 512, (j + 1) * 512)
                            qs = slice(q0 + j * 512, q0 + (j + 1) * 512)
                            nc.tensor.matmul(
                                s_a[:, js], kT[0:64, ka * 128:(ka + 1) * 128],
                                qT[0:64, qs], start=True, stop=True)
                            last_s = nc.tensor.matmul(
                                s_b[:, js], kT[64:128, kb * 128:(kb + 1) * 128],
                                qT[64:128, qs], start=True, stop=True)
                        warmer(256, anchor=last_s)
                        for kt, s_ps in ((ka, s_a), (kb, s_b)):
                            e_r = erp.tile([128, QHW], bf16, tag="e")
                            nc.scalar.activation(out=e_r, in_=s_ps, func=Exp,
                                                 scale=float(SCALE))
                            last_pv = None
                            for j in range(QHW // 512):
                                js = slice(j * 512, (j + 1) * 512)
                                last_pv = nc.tensor.matmul(
                                    pv[:, js], vr[:, kt, :], e_r[:, js],
                                    start=(kt == 0), stop=(kt == LT - 1))
                            warmer(256, anchor=last_pv)

                    pv_sb = pvsb.tile([D + 1, QHW], f32, tag="pvsb")
                    nc.vector.tensor_copy(out=pv_sb, in_=pv)

                    o_all = oallp.tile([128, QHW // 128, D], f32, tag="oall")
                    for qt in range(QHW // 128):
                        ot = trps.tile([128, D + 1], f32, tag="tr")
                        nc.tensor.transpose(
                            ot, pv_sb[:, qt * 128:(qt + 1) * 128],
                            ident[0:D + 1, 0:D + 1])
                        rz = rzp.tile([128, 1], f32, tag="rz")
                        nc.vector.reciprocal(out=rz, in_=ot[:, D:D + 1])
                        nc.vector.tensor_scalar_mul(
                            out=o_all[:, qt, :], in0=ot[:, 0:D], scalar1=rz)
                    nc.sync.dma_start(
                        out=o_d.ap()[b, q0:q0 + QHW, :].rearrange(
                            "(t p) d -> p t d", p=128),
                        in_=o_all)
                while slot < len(next_jobs):
                    next_jobs[slot]()
                    slot += 1

            # batch 0 prologue: loads + transposes, with warmers woven in
            # (also serves as the initial clock warm-up burst)
            jobs0 = prep_load(0)
            for i, job in enumerate(jobs0):
                tr = job()
                if i % 2 == 0 and tr is not None:
                    warmer(256, anchor=tr)
            for _ in range(8):
                warmer()
            for b in range(B_SH):
                nxt = prep_load(b + 1) if b + 1 < B_SH else []
                main(b, nxt)

    nc.finalize()
    return nc


def _get_nc():
    if "nc" not in _cached:
        _cached["nc"] = _build()
    return _cached["nc"]


def kernel(query, key, value):
    from concourse.bass_utils import run_bass_kernel_spmd

    nc = _get_nc()
    query = np.ascontiguousarray(query, dtype=np.float32)
    key = np.ascontiguousarray(key, dtype=np.float32)
    value = np.ascontiguousarray(value, dtype=np.float32)

    in_maps = []
    for c in range(NCORES):
        sl = slice(c * B_SH, (c + 1) * B_SH)
        in_maps.append({
            "query": query[sl], "key": key[sl], "value": value[sl]})

    res = run_bass_kernel_spmd(nc, in_maps, core_ids=list(range(NCORES)))
    out = np.concatenate([r["out"] for r in res.results], axis=0)
    return out
